# revision 14
# baseline (speedup 1.0000x reference)
"""ObjectAttentionBlock2D TRN2 kernel (v2: transposed-output pipeline).

Reference computation (per batch b):
    xf    = x[b].reshape(C, N)                  # C=512, N=128*128=16384
    pf    = proxy[b,:,:,0]                      # [C, K], K=64
    query = Wq @ xf + bq                        # [Ck=256, N]
    keym  = Wk @ pf + bk                        # [Ck, K]
    value = (Wv @ pf + bv).T                    # [K, Cv=256]
    sim   = softmax_k(query.T @ keym / 16)      # [N, K]
    ctx   = sim @ value                         # [N, Cv]
    out   = Wo @ ctx.T + bo                     # [C, N]

Sharding: data-parallel over batch. B=8 images -> 8 NeuronCores, no
collectives. Weights replicated.

Algebra (as v1): Q/K/V/O projections fold into small per-core matrices:
  M     = Wq^T @ keym            [C, K]   sim^T = M^T x
  sbias = (bq/16)^T @ keym       [K, 1]   rides in exp's bias slot
  WVT   = (Wo @ value^T)^T + bo  [K, C]   out = WVT^T en  (bo folds in because
                                          softmax columns sum to 1)

v2 layout change: the second matmul runs TRANSPOSED (pixels on PSUM
partitions): outT[n, c] = e[:,n]^T @ WVT with lhsT = e tile [K=64, 128 px].
The softmax denominator then lands as a per-partition column
(den^T = e^T @ ones via a 1-column matmul, nearly free), so the reciprocal
folds into the PSUM->SBUF fp16 downcast as the per-partition `scale` operand
of ACT/DVE ops -- no broadcast matmul, no separate normalize multiply.

Output is written as OUT_T [N, C] fp16 (host transposes + upcasts); this
halves the dominant out DMA stream vs f32. x is split channel-chunk-wise:
first (4-N8) chunks of 128 channels in fp16, last N8 chunks in fp8 E3M4
(4-bit mantissa); N8 tunes DMA bytes vs accuracy.

Per 512-px period (32 periods/core):
  DMA in: x16 [128,(4-N8),512] f16 + x8 [128,N8,512] f8   (>=512B elems)
  PE: 16 sim MMs (f32r lhsT M, out [64,128]) -> 4 denT MMs (out free 1)
      -> 4 outT MMs (lhsT=e [64,128], rhs=WVT [64,512])
  ACT: one exp over [64,4,128] (bias=sbias, scale=1/16) -> e fp16;
       2 of 4 downcasts (Identity, scale=recipT column)
  DVE: reciprocal [128,4]; 2 of 4 downcasts (tensor_scalar mult)
  DMA out: one [128,4,512] fp16 -> OUT_T
All three DMA streams ride the SP HWDGE queue; x is dispatched PF=2 periods
ahead of the compute that consumes it (software pipeline, tail stage lags one
period so PE never stalls on exp).
"""

from collections import deque

import numpy as np
import ml_dtypes

import concourse.bacc as bacc
import concourse.mybir as mybir
import concourse.tile as tile
from concourse import bass_utils

F32 = mybir.dt.float32
F32R = mybir.dt.float32r
F16 = mybir.dt.float16
F8 = mybir.dt.float8e3

B, C, H, W = 8, 512, 128, 128
N = H * W                    # 16384 pixels per image
CK, CV, K = 256, 256, 64
P = 128                      # SBUF partitions
FP = 512                     # pixels per period (DMA granule)
NT = 4                       # sub-tiles per period
FT = FP // NT                # 128 pixels per sub-tile (= out PSUM partitions)
NP = N // FP                 # 32 periods
CI_CH = C // P               # 4 contraction chunks over C
Q_CH = CK // P               # 2 chunks over Ck
V_CH = CV // P               # 2 chunks over Cv
SCALE = CK ** -0.5           # 1/16
PF = 24                      # x prefetch run-ahead (SP queue depth paces actual issue)

N8 = 4                       # trailing C-chunks of x stored as fp8 E3M4
C16 = CI_CH - N8

_CACHED = None


def _build(n8):
    c16 = CI_CH - n8
    nc = bacc.Bacc("TRN2", target_bir_lowering=False, debug=False)

    X16 = (
        nc.dram_tensor("x16", [c16 * P, N], F16, kind="ExternalInput").ap()
        if c16 else None
    )
    X8 = (
        nc.dram_tensor("x8", [n8 * P, N], F8, kind="ExternalInput").ap()
        if n8 else None
    )
    # pack16[c, :] = [pf(64) | wkT(256) | wvT(256)] in fp16
    PACK16 = nc.dram_tensor("pack16", [C, 576], F16, kind="ExternalInput").ap()
    WQ = nc.dram_tensor("wq", [CK, C], F16, kind="ExternalInput").ap()
    # crow = [bk(256) | bv(256) | ones(256) | bo(512)] as one row
    CROW = nc.dram_tensor("crow", [1, 1280], F32, kind="ExternalInput").ap()
    BQS16 = nc.dram_tensor("bqs16", [P, 2], F16, kind="ExternalInput").ap()
    WOT = nc.dram_tensor("woT", [CV, C], F16, kind="ExternalInput").ap()
    OUTT = nc.dram_tensor("outt", [N, C], F16, kind="ExternalOutput").ap()

    x16_r = X16.rearrange("(co p) n -> p co n", p=P) if c16 else None
    x8_r = X8.rearrange("(co p) n -> p co n", p=P) if n8 else None
    outt_r = OUTT.rearrange("(q t p) c -> p q t c", t=NT, p=P)

    Exp = mybir.ActivationFunctionType.Exp
    Ident = mybir.ActivationFunctionType.Identity

    with tile.TileContext(nc) as tc:
        with tc.tile_pool(name="const", bufs=1) as cp:
            pack = cp.tile([P, CI_CH, 576], F16)
            nc.sync.dma_start(pack, PACK16.rearrange("(co p) q -> p co q", p=P))
            pf = pack[:, :, 0:K]
            wk = pack[:, :, K:K + CK]
            wv = pack[:, :, K + CK:K + CK + CV]
            wq = cp.tile([P, Q_CH, C], F16)
            nc.sync.dma_start(wq, WQ.rearrange("(qo p) c -> p qo c", p=P))
            crow = cp.tile([1, 1280], F32R)
            nc.scalar.dma_start(crow, CROW.bitcast(F32R))
            bk_row = crow[:, 0:CK]
            bv_row = crow[:, CK:CK + CV]
            ones_row = crow[:, 512:768]
            bo_row = crow[:, 768:1280]
            ones_col = cp.tile([K, 1], F16)
            nc.vector.memset(ones_col, 1.0)
            bqs = cp.tile([P, 2], F16)
            nc.scalar.dma_start(bqs, BQS16)
            wo = cp.tile([P, V_CH, C], F16)
            nc.sync.dma_start(wo, WOT.rearrange("(vo p) o -> p vo o", p=P))

            keym = cp.tile([P, Q_CH, K], F16)    # [q-part, q-chunk, k]
            wvt = cp.tile([K, C], F16)          # (Wo @ value^T)^T + bo
            msim = cp.tile([P, CI_CH, K], F16)  # M[c,k] = sum_q Wq[q,c]*keym[q,k]
            sbias = cp.tile([K, 1], F32)         # sum_q (bq/16)[q]*keym[q,k]

            # ---- one-time: keym = Wk @ pf + bk; value2[v,k]; WVT; M; sbias
            with tc.tile_pool(name="setup_ps", bufs=1, space="PSUM") as sps:
                kps = sps.tile([P, Q_CH, K], F32)
                for qi in range(Q_CH):
                    for ci in range(CI_CH):
                        nc.tensor.matmul(
                            kps[:, qi, :],
                            wk[:, ci, qi * P:(qi + 1) * P],
                            pf[:, ci, :],
                            start=(ci == 0), stop=False,
                        )
                    nc.tensor.matmul(
                        kps[:, qi, :],
                        bk_row[:, qi * P:(qi + 1) * P],
                        ones_row[:, :K],
                        start=False, stop=True,
                    )
                nc.vector.tensor_copy(keym, kps)

                mps = sps.tile([P, CI_CH, K], F32)
                for ci in range(CI_CH):
                    for qi in range(Q_CH):
                        nc.tensor.matmul(
                            mps[:, ci, :],
                            wq[:, qi, ci * P:(ci + 1) * P],
                            keym[:, qi, :],
                            start=(qi == 0), stop=(qi == Q_CH - 1),
                        )
                nc.vector.tensor_copy(msim, mps)

                sbps = sps.tile([K, 1], F32)
                for qi in range(Q_CH):
                    nc.tensor.matmul(
                        sbps, keym[:, qi, :], bqs[:, qi:qi + 1],
                        start=(qi == 0), stop=(qi == Q_CH - 1),
                    )
                nc.vector.tensor_copy(sbias, sbps)
                v2ps = sps.tile([P, V_CH, K], F32)
                for vi in range(V_CH):
                    for ci in range(CI_CH):
                        nc.tensor.matmul(
                            v2ps[:, vi, :],
                            wv[:, ci, vi * P:(vi + 1) * P],
                            pf[:, ci, :],
                            start=(ci == 0), stop=False,
                        )
                    nc.tensor.matmul(
                        v2ps[:, vi, :],
                        bv_row[:, vi * P:(vi + 1) * P],
                        ones_row[:, :K],
                        start=False, stop=True,
                    )
                v2sb = cp.tile([P, V_CH, K], F16)
                nc.vector.tensor_copy(v2sb, v2ps)

                wvtps = sps.tile([K, C], F32)
                for vi in range(V_CH):
                    nc.tensor.matmul(
                        wvtps, v2sb[:, vi, :], wo[:, vi, :],
                        start=(vi == 0), stop=False,
                    )
                # += ones[k] * bo[c]: softmax columns sum to 1, so adding
                # bo[c] to every row of WVT realises the +bo of the output.
                nc.tensor.matmul(
                    wvtps, ones_row[:, :K], bo_row,
                    start=False, stop=True,
                )
                nc.vector.tensor_copy(wvt, wvtps)


            # ---- steady-state software pipeline over 512-px periods
            with (
                tc.tile_pool(name="xin16", bufs=(32 if c16 <= 2 else 26)) as xp16,
                tc.tile_pool(name="xin8", bufs=32) as xp8,
                tc.tile_pool(name="esb", bufs=8) as ep,
                tc.tile_pool(name="rsb", bufs=8) as rp,
                tc.tile_pool(name="outsb", bufs=8) as outp,
                tc.tile_pool(name="simps", bufs=2, space="PSUM") as simps,
                tc.tile_pool(name="outps", bufs=5, space="PSUM") as outps,
                tc.tile_pool(name="denps", bufs=1, space="PSUM") as denps,
            ):
                xq = deque()

                def dispatch_x(j):
                    if j >= NP:
                        return
                    n0 = j * FP
                    t16 = t8 = None
                    if c16:
                        t16 = xp16.tile([P, c16, FP], F16, tag="x16")
                        nc.sync.dma_start(t16, x16_r[:, :, n0:n0 + FP])
                    if n8:
                        t8 = xp8.tile([P, n8, FP], F8, tag="x8")
                        nc.sync.dma_start(t8, x8_r[:, :, n0:n0 + FP])
                    xq.append((t16, t8))

                def tail(e_t, j):
                    den = denps.tile([P, NT], F32, tag="den")
                    for t in range(NT):
                        nc.tensor.matmul(
                            den[:, t:t + 1], e_t[:, t * FT:(t + 1) * FT], ones_col,
                            start=True, stop=True,
                        )
                    r = rp.tile([P, NT], F32, tag="r")
                    with nc.allow_low_precision(reason="positive softmax denom"):
                        nc.vector.reciprocal(r, den)
                    o_sb = outp.tile([P, NT, C], F16, tag="osb")
                    for t in range(NT):
                        ops = outps.tile([P, C], F32, tag="ops")
                        nc.tensor.matmul(
                            ops, e_t[:, t * FT:(t + 1) * FT], wvt, start=True, stop=True,
                        )
                        sc = r[:, t:t + 1]
                        # alternate 2/2 and 1/3 ACT/DVE cast split so neither
                        # engine's steady load exceeds ~70% of the DMA period
                        n_act = 2 if (j % 2 == 0) else 1
                        if t < n_act:
                            nc.scalar.activation(o_sb[:, t, :], ops, Ident, scale=sc)
                        else:
                            nc.vector.tensor_scalar_mul(o_sb[:, t, :], ops, sc)
                    # out rides the Pool SWDGE queue: its wait-on-casts must
                    # not block SP's run-ahead x prefetch dispatches. In the
                    # drain (x stream done) SP is free and HWDGE has lower
                    # dispatch latency, so the last outs go there.
                    if j >= NP - 3:
                        nc.sync.dma_start(outt_r[:, j, :, :], o_sb)
                    else:
                        nc.gpsimd.dma_start(outt_r[:, j, :, :], o_sb)

                for j in range(PF):
                    dispatch_x(j)
                pend = deque()
                for ip in range(NP):
                    x16_t, x8_t = xq.popleft()
                    # full-width sim MMs: 4 instructions/period (PE.SEQ
                    # dispatch cost is per-instruction, engine time is not)
                    sim = simps.tile([K, FP], F32, tag="sim")
                    for ci in range(CI_CH):
                        if ci < c16:
                            src = x16_t[:, ci, :]
                        else:
                            src = x8_t[:, ci - c16, :]
                        nc.tensor.matmul(
                            sim, msim[:, ci, :], src,
                            start=(ci == 0), stop=(ci == CI_CH - 1),
                        )
                    e_t = ep.tile([K, FP], F16, tag="e")
                    nc.scalar.activation(e_t, sim, Exp, scale=SCALE, bias=sbias)
                    dispatch_x(ip + PF)
                    if pend:
                        tail(*pend.popleft())
                    pend.append((e_t, ip))
                tail(*pend.popleft())

    nc.compile()
    return nc


def _get_nc():
    global _CACHED
    if _CACHED is None:
        _CACHED = _build(N8)
    return _CACHED


def kernel(x, proxy, Wq, bq, Wk, bk, Wv, bv, Wo, bo, **run_kwargs):
    nc = _get_nc()

    crow = np.concatenate(
        [np.asarray(bk, np.float32).reshape(1, CK),
         np.asarray(bv, np.float32).reshape(1, CV),
         np.ones((1, 256), np.float32),
         np.asarray(bo, np.float32).reshape(1, C)], axis=1)
    w16 = np.concatenate(
        [np.asarray(Wk).T, np.asarray(Wv).T], axis=1
    ).astype(np.float16)
    shared = {
        "woT": np.ascontiguousarray(np.asarray(Wo).T).astype(np.float16),
        "wq": np.ascontiguousarray(np.asarray(Wq)).astype(np.float16),
        "bqs16": np.ascontiguousarray(
            (np.asarray(bq, np.float32) * SCALE).reshape(2, P).T
        ).astype(np.float16),
        "crow": np.ascontiguousarray(crow),
    }
    in_maps = []
    for b in range(B):
        m = dict(shared)
        xf = np.asarray(x[b]).reshape(C, N)
        if C16:
            m["x16"] = np.ascontiguousarray(xf[: C16 * P]).astype(np.float16)
        if N8:
            m["x8"] = np.ascontiguousarray(xf[C16 * P:]).astype(
                ml_dtypes.float8_e3m4
            )
        pf16 = np.asarray(proxy[b, :, :, 0]).astype(np.float16)
        m["pack16"] = np.ascontiguousarray(np.concatenate([pf16, w16], axis=1))
        in_maps.append(m)

    res = bass_utils.run_bass_kernel_spmd(
        nc, in_maps, core_ids=list(range(B)), **run_kwargs
    )
    out = np.stack(
        [
            np.asarray(res.results[b]["outt"]).astype(np.float32).T
            for b in range(B)
        ],
        axis=0,
    )
    if run_kwargs:
        kernel.last_results = res
    return out.reshape(B, C, H, W)
